# revision 13
# baseline (speedup 1.0000x reference)
"""Trainium2 Bass kernel for CW-Semi-PU contrastive loss.

Strategy
--------
All four loss terms are built from per-anchor (row-i) weighted reductions
over the partner axis j of elementwise-derived matrices:

  Z_i = sum_j exp(l_ij - C)            (l = sim/TAU, C = const shift)
  P_i = sum_j pos_j  * exp(l_ij - C)
  A_i = sum_j posA_j * l_ij            (posA = pos * alpha)
  R_i = sum_j rnB_j  * w_ij * exp(l_ij - C)   (rnB = rn * beta)
  U_i = sum_j u_j    * w_ij * exp(l_ij - C)

On the device we keep j on the SBUF partition axis (inputs are laid out
transposed on the host as part of sharding), so each reduction is a small
accumulating matmul with the per-j weight vectors as the stationary operand.
Per 128-row j-tile: one ACT exp, one DVE scale, one DVE multiply and six
N=512 matmuls. The kernel is DMA-bound (~48MB/core).

The anchor dim (8192 rows) is sharded across 8 cores: core c computes
Z/P/A/R/U for anchors [1024c, 1024(c+1)).  The O(B) finalization
(diagonal exclusion, logZ, count normalization, nnPU debias, mean) runs
on the host in float64.
"""

import numpy as np
import ml_dtypes

TAU = 0.07
LAMBDA_RN = 1.0
LAMBDA_U = 1.0
BETA_FLOOR = 0.0
B = 8192
NCORES = 8
COLS = B // NCORES      # anchors per core (free axis of matmuls)
PT = 128                # partition tile (j)
JT = B // PT            # number of j tiles
# Work in shifted logit units lS = (sim - SHIFT)/TAU = l - 60 so that
# exp(lS) stays in fp32/bf16 range (row max logit ~ 74) and no activation
# bias constant is needed.  The shift cancels in (l - logZ) so only the
# host-side logZ uses the shifted convention consistently.
SIM_SHIFT = np.float32(60.0 * TAU)

BF16 = ml_dtypes.bfloat16

_CACHE = {}


def _build():
    """Build + schedule the single-core SPMD bass program."""
    from contextlib import ExitStack

    import concourse.bacc as bacc
    import concourse.tile as tile
    import concourse.mybir as mybir

    f32 = mybir.dt.float32
    bf16 = mybir.dt.bfloat16

    nc = bacc.Bacc(
        "TRN2",
        target_bir_lowering=False,
        debug=False,
        num_devices=NCORES,
    )

    simT = nc.dram_tensor("simT", [B, COLS], f32, kind="ExternalInput").ap()
    wT = nc.dram_tensor("wT", [B, COLS], bf16, kind="ExternalInput").ap()
    wzp = nc.dram_tensor("wzp", [PT, JT, 2], bf16, kind="ExternalInput").ap()
    wa = nc.dram_tensor("wa", [PT, JT, 1], bf16, kind="ExternalInput").ap()
    wru = nc.dram_tensor("wru", [PT, JT, 2], bf16, kind="ExternalInput").ap()
    acc_out = nc.dram_tensor("acc", [5, COLS], f32, kind="ExternalOutput").ap()

    H = COLS // 512  # matmul free-dim chunks per reduction

    with tile.TileContext(nc) as tc:
        with ExitStack() as ctx:
            const_pool = ctx.enter_context(tc.tile_pool(name="const", bufs=1))
            in_pool = ctx.enter_context(tc.tile_pool(name="inp", bufs=8))
            mid_pool = ctx.enter_context(tc.tile_pool(name="mid", bufs=4))
            psum_pool = ctx.enter_context(
                tc.tile_pool(name="acc", bufs=1, space="PSUM")
            )
            out_pool = ctx.enter_context(tc.tile_pool(name="out", bufs=1))

            # Keep the two big streaming loads on separate HWDGE rings
            # (sync + scalar); constants go via SWDGE to stay off both.
            wzp_t = const_pool.tile([PT, JT, 2], bf16)
            nc.gpsimd.dma_start(wzp_t[:], wzp[:])
            wa_t = const_pool.tile([PT, JT, 1], bf16)
            nc.gpsimd.dma_start(wa_t[:], wa[:])
            wru_t = const_pool.tile([PT, JT, 2], bf16)
            nc.gpsimd.dma_start(wru_t[:], wru[:])

            acc_zp = psum_pool.tile([2, COLS], f32)
            acc_a = psum_pool.tile([1, COLS], f32)
            acc_ru = psum_pool.tile([2, COLS], f32)

            for jj in range(JT):
                s_t = in_pool.tile([PT, COLS], f32, tag="sT")
                nc.sync.dma_start(s_t[:], simT[jj * PT:(jj + 1) * PT, :])
                w_t = in_pool.tile([PT, COLS], bf16, tag="wT")
                # SWDGE: GpSimd is otherwise idle so these dispatch promptly;
                # issuing them from sync/scalar makes them queue behind the
                # busy engine's instruction stream and starves the pipeline.
                nc.gpsimd.dma_start(w_t[:], wT[jj * PT:(jj + 1) * PT, :])

                e_t = mid_pool.tile([PT, COLS], bf16, tag="eT")
                nc.scalar.activation(
                    e_t[:],
                    s_t[:],
                    mybir.ActivationFunctionType.Exp,
                    bias=0.0,
                    scale=1.0 / TAU,
                )
                l_t = mid_pool.tile([PT, COLS], bf16, tag="lT")
                nc.vector.tensor_scalar_mul(l_t[:], s_t[:], 1.0 / TAU)
                we_t = mid_pool.tile([PT, COLS], bf16, tag="weT")
                nc.vector.tensor_mul(we_t[:], w_t[:], e_t[:])

                flags = dict(start=jj == 0, stop=jj == JT - 1)
                for h in range(H):
                    sl = slice(h * 512, (h + 1) * 512)
                    nc.tensor.matmul(
                        acc_zp[:, sl], wzp_t[:, jj, :], e_t[:, sl], **flags
                    )
                    nc.tensor.matmul(
                        acc_a[:, sl], wa_t[:, jj, :], l_t[:, sl], **flags
                    )
                    nc.tensor.matmul(
                        acc_ru[:, sl], wru_t[:, jj, :], we_t[:, sl], **flags
                    )

            # Engine APs must start at a partition quadrant boundary, so use
            # one SBUF tile per accumulator and three small DMAs out.
            out_zp = out_pool.tile([2, COLS], f32, tag="ozp")
            nc.vector.tensor_copy(out_zp[:], acc_zp[:])
            out_a = out_pool.tile([1, COLS], f32, tag="oa")
            nc.scalar.copy(out_a[:], acc_a[:])
            out_ru = out_pool.tile([2, COLS], f32, tag="oru")
            nc.vector.tensor_copy(out_ru[:], acc_ru[:])
            nc.gpsimd.dma_start(acc_out[0:2, :], out_zp[:])
            nc.gpsimd.dma_start(acc_out[2:3, :], out_a[:])
            nc.gpsimd.dma_start(acc_out[3:5, :], out_ru[:])

    nc.compile()
    return nc


def _get_nc():
    if "nc" not in _CACHE:
        _CACHE["nc"] = _build()
    return _CACHE["nc"]


def _prep_in_maps(sim_matrix, pu_weights, pu_labels, alphas, betas):
    """Host-side sharding: transpose slices (j on partitions), build the
    per-j stationary weight vectors."""
    inv = np.float32(1.0 / TAU)

    lab = np.asarray(pu_labels)
    pos = lab == 1
    rn = lab == -1
    ul = lab == 0

    alphas = np.asarray(alphas, dtype=np.float32)
    betas = np.asarray(betas, dtype=np.float32)

    ones16 = np.ones(B, dtype=BF16)
    pos16 = pos.astype(np.float32).astype(BF16)
    u16 = ul.astype(np.float32).astype(BF16)
    posalpha16 = np.where(pos, alphas, 0.0).astype(np.float32).astype(BF16)
    rnbeta16 = np.where(rn, betas, 0.0).astype(np.float32).astype(BF16)

    def fold(vecs):
        # [B, k] -> [128, 64, k] with element (p, jj, v) = vec_v[jj*128 + p]
        m = np.stack(vecs, axis=1)
        return np.ascontiguousarray(
            m.reshape(JT, PT, len(vecs)).transpose(1, 0, 2)
        )

    wzp = fold([ones16, pos16])
    wa = fold([posalpha16])
    wru = fold([rnbeta16, u16])

    sim = np.asarray(sim_matrix, dtype=np.float32)
    w = np.asarray(pu_weights, dtype=np.float32)

    in_maps = []
    for c in range(NCORES):
        sl = slice(c * COLS, (c + 1) * COLS)
        sim_sh = np.ascontiguousarray(sim[sl, :].T)
        sim_sh -= SIM_SHIFT
        in_maps.append(
            {
                "simT": sim_sh,
                "wT": w[sl, :].T.astype(BF16, order="C"),
                "wzp": wzp,
                "wa": wa,
                "wru": wru,
            }
        )
    return in_maps, dict(
        pos=pos, rn=rn, ul=ul,
        posalpha16=posalpha16, rnbeta16=rnbeta16,
        alphas=alphas, betas=betas,
    )


def _epilogue(accs, sim_matrix, pu_weights, pi_a, meta):
    """O(B) finalization in float64 on host: diagonal exclusion, logZ,
    count normalization, debiased nnPU term, mean."""
    Z = np.concatenate([a[0] for a in accs]).astype(np.float64)
    P = np.concatenate([a[1] for a in accs]).astype(np.float64)
    A = np.concatenate([a[2] for a in accs]).astype(np.float64)
    R = np.concatenate([a[3] for a in accs]).astype(np.float64)
    U = np.concatenate([a[4] for a in accs]).astype(np.float64)

    pos = meta["pos"]
    rn = meta["rn"]
    ul = meta["ul"]

    sim_d = np.ascontiguousarray(np.diagonal(np.asarray(sim_matrix))).astype(
        np.float32
    )
    w_d = np.ascontiguousarray(np.diagonal(np.asarray(pu_weights))).astype(
        np.float32
    )

    # Replicate device arithmetic for the diagonal terms so the exclusion
    # matches what the matmuls actually accumulated (bf16 rounding included).
    l_diag_f32 = (sim_d - SIM_SHIFT) * np.float32(1.0 / TAU)
    e_f32 = np.exp(l_diag_f32.astype(np.float64)).astype(np.float32)
    e_b = e_f32.astype(BF16).astype(np.float64)
    l_b = l_diag_f32.astype(BF16).astype(np.float64)
    w_b16 = w_d.astype(BF16)
    we_b = (
        (w_b16.astype(np.float32) * e_f32.astype(BF16).astype(np.float32))
        .astype(BF16)
        .astype(np.float64)
    )

    posalpha = meta["posalpha16"].astype(np.float64)
    rnbeta = meta["rnbeta16"].astype(np.float64)

    pos_f = pos.astype(np.float64)
    rn_f = rn.astype(np.float64)
    u_f = ul.astype(np.float64)

    Z_ex = Z - e_b
    P_ex = P - pos_f * e_b
    A_ex = A - posalpha * l_b
    R_ex = R - rnbeta * we_b
    U_ex = U - u_f * we_b

    cnt_pos = pos.sum() - pos_f
    cnt_rn = rn.sum() - rn_f
    cnt_u = ul.sum() - u_f
    has_pos = cnt_pos > 0
    has_rn = cnt_rn > 0
    has_u = cnt_u > 0

    SA = posalpha.sum() - posalpha  # sum_{j != i} pos_j * alpha_j (bf16 vals)

    # logZ in the shifted convention; matches A_ex which is in lS units,
    # and the shift cancels in (l - logZ).
    logZ = np.log(Z_ex)

    L_pos = -(A_ex - SA * logZ) / np.maximum(cnt_pos, 1.0)
    L_pos = np.where(has_pos, L_pos, 0.0)

    L_rn = (R_ex / Z_ex) / np.maximum(cnt_rn, 1.0)
    L_rn = np.where(has_rn, L_rn, 0.0)

    E_U = (U_ex / Z_ex) / np.maximum(cnt_u, 1.0)
    E_P = (P_ex / Z_ex) / np.maximum(cnt_pos, 1.0)
    pi = np.clip(np.asarray(pi_a, dtype=np.float64), 1e-4, 0.5)
    debiased = (E_U - pi * E_P) / (1.0 - pi + 1e-8)
    L_u = np.maximum(debiased, BETA_FLOOR)
    L_u = np.where(has_u & has_pos, L_u, 0.0)

    total = np.mean(L_pos + LAMBDA_RN * L_rn + LAMBDA_U * L_u)
    return np.array(total, dtype=np.float32)


def _run(sim_matrix, alphas, betas, pi_a, pu_weights, pu_labels, trace=False):
    from concourse.bass_utils import run_bass_kernel_spmd

    nc = _get_nc()
    in_maps, meta = _prep_in_maps(sim_matrix, pu_weights, pu_labels, alphas, betas)
    res = run_bass_kernel_spmd(
        nc, in_maps, core_ids=list(range(NCORES)), trace=trace
    )
    accs = [res.results[c]["acc"] for c in range(NCORES)]
    loss = _epilogue(accs, sim_matrix, pu_weights, pi_a, meta)
    return loss, res


def kernel(sim_matrix, alphas, betas, pi_a, pu_weights, pu_labels):
    loss, _ = _run(sim_matrix, alphas, betas, pi_a, pu_weights, pu_labels)
    return loss


# revision 16
# speedup vs baseline: 1.2756x; 1.2756x over previous
"""Trainium2 Bass kernel for CW-Semi-PU contrastive loss.

Strategy
--------
All four loss terms are built from per-anchor (row-i) weighted reductions
over the partner axis j of elementwise-derived matrices:

  Z_i = sum_j exp(l_ij - C)            (l = sim/TAU, C = const shift)
  P_i = sum_j pos_j  * exp(l_ij - C)
  A_i = sum_j posA_j * l_ij            (posA = pos * alpha)
  R_i = sum_j rnB_j  * w_ij * exp(l_ij - C)   (rnB = rn * beta)
  U_i = sum_j u_j    * w_ij * exp(l_ij - C)

On the device we keep j on the SBUF partition axis (inputs are laid out
transposed on the host as part of sharding), so each reduction is a small
accumulating matmul with the per-j weight vectors as the stationary operand.
Per 128-row j-tile: one ACT exp, one DVE scale, one DVE multiply and six
N=512 matmuls. The kernel is DMA-bound (~48MB/core).

The anchor dim (8192 rows) is sharded across 8 cores: core c computes
Z/P/A/R/U for anchors [1024c, 1024(c+1)).  The O(B) finalization
(diagonal exclusion, logZ, count normalization, nnPU debias, mean) runs
on the host in float64.
"""

import numpy as np
import ml_dtypes

TAU = 0.07
LAMBDA_RN = 1.0
LAMBDA_U = 1.0
BETA_FLOOR = 0.0
B = 8192
NCORES = 8
COLS = B // NCORES      # anchors per core (free axis of matmuls)
PT = 128                # partition tile (j)
JT = B // PT            # number of j tiles
# Work in shifted logit units lS = (sim - SHIFT)/TAU = l - 60 so that
# exp(lS) stays in fp32/bf16 range (row max logit ~ 74) and no activation
# bias constant is needed.  The shift cancels in (l - logZ) so only the
# host-side logZ uses the shifted convention consistently.
SIM_SHIFT = np.float32(60.0 * TAU)

BF16 = ml_dtypes.bfloat16
FP8 = ml_dtypes.float8_e4m3

_CACHE = {}


def _build():
    """Build + schedule the single-core SPMD bass program."""
    from contextlib import ExitStack

    import concourse.bacc as bacc
    import concourse.tile as tile
    import concourse.mybir as mybir

    f32 = mybir.dt.float32
    f32r = mybir.dt.float32r
    bf16 = mybir.dt.bfloat16
    fp8 = mybir.dt.float8e4

    nc = bacc.Bacc(
        "TRN2",
        target_bir_lowering=False,
        debug=False,
        num_devices=NCORES,
    )

    # simT is typed float32r (same bits as f32): the A-term matmul consumes
    # it directly in the PE's fast fp32r mode; ACT reads it bitcast as f32.
    simT = nc.dram_tensor("simT", [B, COLS], f32r, kind="ExternalInput").ap()
    wT = nc.dram_tensor("wT", [B, COLS], fp8, kind="ExternalInput").ap()
    wzp = nc.dram_tensor("wzp", [PT, JT, 2], bf16, kind="ExternalInput").ap()
    wa = nc.dram_tensor("wa", [PT, JT, 1], f32r, kind="ExternalInput").ap()
    wru = nc.dram_tensor("wru", [PT, JT, 2], bf16, kind="ExternalInput").ap()
    acc_out = nc.dram_tensor("acc", [5, COLS], f32, kind="ExternalOutput").ap()

    H = COLS // 512  # matmul free-dim chunks per reduction

    with tile.TileContext(nc) as tc:
        with ExitStack() as ctx:
            const_pool = ctx.enter_context(tc.tile_pool(name="const", bufs=1))
            in_pool = ctx.enter_context(tc.tile_pool(name="inp", bufs=8))
            mid_pool = ctx.enter_context(tc.tile_pool(name="mid", bufs=4))
            psum_pool = ctx.enter_context(
                tc.tile_pool(name="acc", bufs=1, space="PSUM")
            )
            out_pool = ctx.enter_context(tc.tile_pool(name="out", bufs=1))

            # Keep the two big streaming loads on separate HWDGE rings
            # (sync + scalar); constants go via SWDGE to stay off both.
            wzp_t = const_pool.tile([PT, JT, 2], bf16)
            nc.gpsimd.dma_start(wzp_t[:], wzp[:])
            wa_t = const_pool.tile([PT, JT, 1], f32r)
            nc.gpsimd.dma_start(wa_t[:], wa[:])
            wru_t = const_pool.tile([PT, JT, 2], bf16)
            nc.gpsimd.dma_start(wru_t[:], wru[:])

            acc_zp = psum_pool.tile([2, COLS], f32)
            acc_a = psum_pool.tile([1, COLS], f32)
            acc_ru = psum_pool.tile([2, COLS], f32)

            for jj in range(JT):
                s_t = in_pool.tile([PT, COLS], f32r, tag="sT")
                nc.sync.dma_start(s_t[:], simT[jj * PT:(jj + 1) * PT, :])
                w_t = in_pool.tile([PT, COLS], fp8, tag="wT")
                nc.sync.dma_start(w_t[:], wT[jj * PT:(jj + 1) * PT, :])

                e_t = mid_pool.tile([PT, COLS], bf16, tag="eT")
                nc.scalar.activation(
                    e_t[:],
                    s_t[:].bitcast(f32),
                    mybir.ActivationFunctionType.Exp,
                    bias=0.0,
                    scale=1.0 / TAU,
                )
                we_t = mid_pool.tile([PT, COLS], bf16, tag="weT")
                nc.vector.tensor_mul(we_t[:], w_t[:], e_t[:])

                flags = dict(start=jj == 0, stop=jj == JT - 1)
                for h in range(H):
                    sl = slice(h * 512, (h + 1) * 512)
                    nc.tensor.matmul(
                        acc_zp[:, sl], wzp_t[:, jj, :], e_t[:, sl], **flags
                    )
                    nc.tensor.matmul(
                        acc_a[:, sl], wa_t[:, jj, :], s_t[:, sl], **flags
                    )
                    nc.tensor.matmul(
                        acc_ru[:, sl], wru_t[:, jj, :], we_t[:, sl], **flags
                    )

            # Engine APs must start at a partition quadrant boundary, so use
            # one SBUF tile per accumulator and three small DMAs out.
            out_zp = out_pool.tile([2, COLS], f32, tag="ozp")
            nc.vector.tensor_copy(out_zp[:], acc_zp[:])
            out_a = out_pool.tile([1, COLS], f32, tag="oa")
            nc.scalar.copy(out_a[:], acc_a[:])
            out_ru = out_pool.tile([2, COLS], f32, tag="oru")
            nc.vector.tensor_copy(out_ru[:], acc_ru[:])
            nc.gpsimd.dma_start(acc_out[0:2, :], out_zp[:])
            nc.gpsimd.dma_start(acc_out[2:3, :], out_a[:])
            nc.gpsimd.dma_start(acc_out[3:5, :], out_ru[:])

    nc.compile()
    return nc


def _get_nc():
    if "nc" not in _CACHE:
        _CACHE["nc"] = _build()
    return _CACHE["nc"]


def _prep_in_maps(sim_matrix, pu_weights, pu_labels, alphas, betas):
    """Host-side sharding: transpose slices (j on partitions), build the
    per-j stationary weight vectors."""
    inv = np.float32(1.0 / TAU)

    lab = np.asarray(pu_labels)
    pos = lab == 1
    rn = lab == -1
    ul = lab == 0

    alphas = np.asarray(alphas, dtype=np.float32)
    betas = np.asarray(betas, dtype=np.float32)

    ones16 = np.ones(B, dtype=BF16)
    pos16 = pos.astype(np.float32).astype(BF16)
    u16 = ul.astype(np.float32).astype(BF16)
    posalpha = np.where(pos, alphas, 0.0).astype(np.float32)
    rnbeta16 = np.where(rn, betas, 0.0).astype(np.float32).astype(BF16)

    def fold(vecs):
        # [B, k] -> [128, 64, k] with element (p, jj, v) = vec_v[jj*128 + p]
        m = np.stack(vecs, axis=1)
        return np.ascontiguousarray(
            m.reshape(JT, PT, len(vecs)).transpose(1, 0, 2)
        )

    wzp = fold([ones16, pos16])
    wa = np.ascontiguousarray(
        posalpha.reshape(JT, PT, 1).transpose(1, 0, 2)
    )
    wru = fold([rnbeta16, u16])

    sim = np.asarray(sim_matrix, dtype=np.float32)
    w = np.asarray(pu_weights, dtype=np.float32)

    in_maps = []
    for c in range(NCORES):
        sl = slice(c * COLS, (c + 1) * COLS)
        sim_sh = np.ascontiguousarray(sim[sl, :].T)
        sim_sh -= SIM_SHIFT
        in_maps.append(
            {
                "simT": sim_sh,
                "wT": w[sl, :].T.astype(FP8, order="C"),
                "wzp": wzp,
                "wa": wa,
                "wru": wru,
            }
        )
    return in_maps, dict(
        pos=pos, rn=rn, ul=ul,
        posalpha=posalpha, rnbeta16=rnbeta16,
        alphas=alphas, betas=betas,
    )


def _epilogue(accs, sim_matrix, pu_weights, pi_a, meta):
    """O(B) finalization in float64 on host: diagonal exclusion, logZ,
    count normalization, debiased nnPU term, mean."""
    Z = np.concatenate([a[0] for a in accs]).astype(np.float64)
    P = np.concatenate([a[1] for a in accs]).astype(np.float64)
    # The A matmul consumes raw shifted sim (not /TAU); rescale here.
    A = np.concatenate([a[2] for a in accs]).astype(np.float64) / TAU
    R = np.concatenate([a[3] for a in accs]).astype(np.float64)
    U = np.concatenate([a[4] for a in accs]).astype(np.float64)

    pos = meta["pos"]
    rn = meta["rn"]
    ul = meta["ul"]

    sim_d = np.ascontiguousarray(np.diagonal(np.asarray(sim_matrix))).astype(
        np.float32
    )
    w_d = np.ascontiguousarray(np.diagonal(np.asarray(pu_weights))).astype(
        np.float32
    )

    # Replicate device arithmetic for the diagonal terms so the exclusion
    # matches what the matmuls actually accumulated (bf16 rounding included).
    l_diag_f32 = (sim_d - SIM_SHIFT) * np.float32(1.0 / TAU)
    e_f32 = np.exp(l_diag_f32.astype(np.float64)).astype(np.float32)
    e_b = e_f32.astype(BF16).astype(np.float64)
    l_b = l_diag_f32.astype(np.float64)
    w_8 = w_d.astype(FP8)
    we_b = (
        (w_8.astype(np.float32) * e_f32.astype(BF16).astype(np.float32))
        .astype(BF16)
        .astype(np.float64)
    )

    posalpha = meta["posalpha"].astype(np.float64)
    rnbeta = meta["rnbeta16"].astype(np.float64)

    pos_f = pos.astype(np.float64)
    rn_f = rn.astype(np.float64)
    u_f = ul.astype(np.float64)

    Z_ex = Z - e_b
    P_ex = P - pos_f * e_b
    A_ex = A - posalpha * l_b
    R_ex = R - rnbeta * we_b
    U_ex = U - u_f * we_b

    cnt_pos = pos.sum() - pos_f
    cnt_rn = rn.sum() - rn_f
    cnt_u = ul.sum() - u_f
    has_pos = cnt_pos > 0
    has_rn = cnt_rn > 0
    has_u = cnt_u > 0

    SA = posalpha.sum() - posalpha  # sum_{j != i} pos_j * alpha_j (bf16 vals)

    # logZ in the shifted convention; matches A_ex which is in lS units,
    # and the shift cancels in (l - logZ).
    logZ = np.log(Z_ex)

    L_pos = -(A_ex - SA * logZ) / np.maximum(cnt_pos, 1.0)
    L_pos = np.where(has_pos, L_pos, 0.0)

    L_rn = (R_ex / Z_ex) / np.maximum(cnt_rn, 1.0)
    L_rn = np.where(has_rn, L_rn, 0.0)

    E_U = (U_ex / Z_ex) / np.maximum(cnt_u, 1.0)
    E_P = (P_ex / Z_ex) / np.maximum(cnt_pos, 1.0)
    pi = np.clip(np.asarray(pi_a, dtype=np.float64), 1e-4, 0.5)
    debiased = (E_U - pi * E_P) / (1.0 - pi + 1e-8)
    L_u = np.maximum(debiased, BETA_FLOOR)
    L_u = np.where(has_u & has_pos, L_u, 0.0)

    total = np.mean(L_pos + LAMBDA_RN * L_rn + LAMBDA_U * L_u)
    return np.array(total, dtype=np.float32)


def _run(sim_matrix, alphas, betas, pi_a, pu_weights, pu_labels, trace=False):
    from concourse.bass_utils import run_bass_kernel_spmd

    nc = _get_nc()
    in_maps, meta = _prep_in_maps(sim_matrix, pu_weights, pu_labels, alphas, betas)
    res = run_bass_kernel_spmd(
        nc, in_maps, core_ids=list(range(NCORES)), trace=trace
    )
    accs = [res.results[c]["acc"] for c in range(NCORES)]
    loss = _epilogue(accs, sim_matrix, pu_weights, pi_a, meta)
    return loss, res


def kernel(sim_matrix, alphas, betas, pi_a, pu_weights, pu_labels):
    loss, _ = _run(sim_matrix, alphas, betas, pi_a, pu_weights, pu_labels)
    return loss
